# revision 7
# baseline (speedup 1.0000x reference)
"""Trainium2 Bass kernel for MinimalRNNCell: h_t = x_t@K + h_{t-1}@R, outputs all h_t.

Shapes: x [128, 1024, 512], h0 [128, 512], K [512, 512], R [512, 512].
Sharding: TIME-sharded across 8 cores (each core owns a 128-step chunk with the
full batch of 128), exploiting linearity of the recurrence:
    h_{t0+i} = p_i + g @ R^{i+1}
where p is the zero-initialized local scan of the chunk and g = h_{t0-1} is the
chunk-entry state, obtained from a tiny 8-step boundary chain on the host.
This keeps every matmul full-tile (the batch-sharded alternative leaves only
16 batch rows per core, making every recurrence matmul issue-bound).

Launch 1 (per core): xk = x_chunk @ K fused into the 128-step local scan,
  all in u-major ("hT") orientation; writes p to HBM.
Host: boundary chain g_{c+1} = p_last_c + g_c @ R^128 (fp64), plus
  d_{c,j} = g_c @ (R^16)^j for j=0..7.
Launch 2 (per core): out_t = transpose(p_t) + d_j @ R^{m+1} for t=16j+m,
  with R^1..R^16 resident in SBUF; writes final [128, 128, 512] chunk.

All matmuls are true fp32 (4 cycles/row on the PE): the recurrence amplifies
matmul rounding error ~1000x over the sequence, so bf16/tf32 are not usable.
"""

import sys

if "/opt/trn_rl_repo" not in sys.path:
    sys.path.insert(0, "/opt/trn_rl_repo")

import numpy as np

import concourse.bacc as bacc
import concourse.bass as bass
import concourse.mybir as mybir
from concourse.bass_utils import run_bass_kernel_spmd
from concourse.tile import TileContext

N_CORES = 8
B, T, D, U = 128, 1024, 512, 512
L = T // N_CORES  # 128 timesteps per core
NG = L // 4  # 4-step groups in the local scan
FP32 = mybir.dt.float32


def _patch_tile_drain(max_waits=1):
    """Walrus in this image rejects >1 sem wait on a CTRL (Drain) inst; split
    the TileContext tail drain into one drain per pending proc."""
    from concourse import tile
    from concourse.vector_clock import ScopedClock, VectorClock

    if getattr(tile.TileContext, "_drain_patched", False):
        return

    def _drain_and_barrier(self, tick_clock, wait_clock):
        nc = self.nc
        gc = tick_clock.global_clock
        nonzero = [(i, gc[i]) for i in range(len(gc)) if gc[i] > 0]
        for g in range(0, len(nonzero), max_waits):
            sub = VectorClock()
            for i, t in nonzero[g : g + max_waits]:
                sub.require_at_least(i, t)
            d = nc.sync.drain()
            wait_clock.add_sem_waits(d.ins, ScopedClock({None: sub}))
        nc.all_engine_barrier()
        assert self.sems is not None
        popped = nc._tile_sem_poison_stack.pop()
        assert popped is self._sem_poison
        nc.clear_and_free_semaphores(list(self.sems.allocated().values()))
        nc.all_engine_barrier()

    tile.TileContext._drain_and_barrier = _drain_and_barrier
    tile.TileContext._drain_patched = True


def build_scan_program():
    """Launch 1: fused xk GEMM + local scan, u-major. Per core:
    in:  xt [4dblk, 128d, 128t, 128b] (host-pretransposed x chunk),
         K [512, 512], R [512, 512]
    out: p [128t, 4ublk, 128u, 128b]  (hT tiles of the zero-init local scan)
    """
    nc = bacc.Bacc("TRN2", target_bir_lowering=False, debug=False, num_devices=N_CORES)
    xt_d = nc.dram_tensor("xt", [4, 128, L, B], FP32, kind="ExternalInput")
    k_d = nc.dram_tensor("k", [D, U], FP32, kind="ExternalInput")
    r_d = nc.dram_tensor("r", [U, U], FP32, kind="ExternalInput")
    p_d = nc.dram_tensor("p", [L, 4, 128, B], FP32, kind="ExternalOutput")

    with TileContext(nc) as tc:
        with (
            tc.tile_pool(name="const", bufs=1) as cpool,
            tc.tile_pool(name="xt", bufs=3) as xtpool,
            tc.tile_pool(name="ht", bufs=4) as htpool,
            tc.tile_pool(name="psum", bufs=2, space="PSUM") as ppool,
        ):
            k_sb = []
            r_sb = []
            for i in range(4):
                kt = cpool.tile([128, U], FP32, tag=f"k{i}")
                nc.sync.dma_start(out=kt[:], in_=k_d[128 * i : 128 * (i + 1), :])
                k_sb.append(kt)
                rt = cpool.tile([128, U], FP32, tag=f"r{i}")
                nc.sync.dma_start(out=rt[:], in_=r_d[128 * i : 128 * (i + 1), :])
                r_sb.append(rt)
            h_zero = []
            for i in range(4):
                z = cpool.tile([128, B], FP32, tag=f"z{i}")
                nc.gpsimd.memset(z[:], 0.0)
                h_zero.append(z)

            def load_group(g):
                xt = [xtpool.tile([128, 4, 128], FP32, tag=f"xt{d}", name=f"xt{d}") for d in range(4)]
                for d in range(4):
                    nc.sync.dma_start(out=xt[d][:], in_=xt_d[d, :, 4 * g : 4 * g + 4, :])
                return xt

            h_prev = h_zero
            xt_cur = load_group(0)
            for g in range(NG):
                xt_next = load_group(g + 1) if g + 1 < NG else None
                ph = [ppool.tile([128, 512], FP32, tag=f"ph{u}", name=f"ph{u}") for u in range(4)]
                for u in range(4):
                    for d in range(4):
                        nc.tensor.matmul(
                            ph[u][:],
                            k_sb[d][:, 128 * u : 128 * (u + 1)],
                            xt_cur[d][:],
                            start=(d == 0),
                            stop=False,
                        )
                for s in range(4):
                    i = 4 * g + s
                    sl = slice(128 * s, 128 * (s + 1))
                    h_new = []
                    for u in range(4):
                        for kb in range(4):
                            nc.tensor.matmul(
                                ph[u][:, sl],
                                r_sb[kb][:, 128 * u : 128 * (u + 1)],
                                h_prev[kb][:],
                                start=False,
                                stop=(kb == 3),
                            )
                        ht = htpool.tile([128, B], FP32, tag=f"ht{u}", name=f"ht{u}")
                        nc.vector.tensor_copy(ht[:], ph[u][:, sl])
                        nc.sync.dma_start(out=p_d[i, u], in_=ht[:])
                        h_new.append(ht)
                    h_prev = h_new
                xt_cur = xt_next
    nc.compile()
    return nc


def build_correct_program():
    """Launch 2: outT_t = p_t + (d_j @ R^{m+1})^T for t=16j+m, all u-major. Per core:
    in:  p [128t, 4ublk, 128u, 128b], dT [8j, 4kblk, 128u', 128b],
         rs [4kblk, 128u', 16*512 (m-major)]
    out: o [128t, 4ublk, 128u, 128b]   (host transposes to [b, t, u])
    """
    nc = bacc.Bacc("TRN2", target_bir_lowering=False, debug=False, num_devices=N_CORES)
    p_d = nc.dram_tensor("p", [L, 4, 128, B], FP32, kind="ExternalInput")
    dt_d = nc.dram_tensor("dT", [8, 4, 128, B], FP32, kind="ExternalInput")
    rs_d = nc.dram_tensor("rs", [4, 128, 16 * U], FP32, kind="ExternalInput")
    o_d = nc.dram_tensor("o", [L, 4, 128, B], FP32, kind="ExternalOutput")

    with TileContext(nc) as tc:
        with (
            tc.tile_pool(name="const", bufs=1) as cpool,
            tc.tile_pool(name="rs", bufs=2) as rspool,
            tc.tile_pool(name="pin", bufs=6) as pinpool,
            tc.tile_pool(name="os", bufs=6) as ospool,
            tc.tile_pool(name="psum", bufs=2, space="PSUM") as ppool,
        ):
            dt_sb = {}
            for j in range(8):
                for kb in range(4):
                    dtt = cpool.tile([128, B], FP32, tag=f"dt{j}_{kb}", name=f"dt{j}_{kb}")
                    nc.sync.dma_start(out=dtt[:], in_=dt_d[j, kb])
                    dt_sb[(j, kb)] = dtt

            def load_rs(m):
                rs = [rspool.tile([128, U], FP32, tag=f"rs{kb}", name=f"rs{kb}") for kb in range(4)]
                for kb in range(4):
                    nc.sync.dma_start(out=rs[kb][:], in_=rs_d[kb, :, m * U : (m + 1) * U])
                return rs

            rs_cur = load_rs(0)
            for m in range(16):
                rs_next = load_rs(m + 1) if m + 1 < 16 else None
                for j in range(8):
                    t = 16 * j + m
                    pin = [pinpool.tile([128, B], FP32, tag=f"pin{u}", name=f"pin{u}") for u in range(4)]
                    for u in range(4):
                        nc.sync.dma_start(out=pin[u][:], in_=p_d[t, u])
                    po = ppool.tile([128, U], FP32, tag=f"po{j % 2}", name=f"po{j % 2}")
                    for u in range(4):
                        for kb in range(4):
                            nc.tensor.matmul(
                                po[:, 128 * u : 128 * (u + 1)],
                                rs_cur[kb][:, 128 * u : 128 * (u + 1)],
                                dt_sb[(j, kb)][:],
                                start=(kb == 0),
                                stop=(kb == 3),
                            )
                    osb = ospool.tile([128, U], FP32, tag="os", name="os")
                    for u in range(4):
                        sl = slice(128 * u, 128 * (u + 1))
                        nc.vector.tensor_add(osb[:, sl], po[:, sl], pin[u][:])
                    for u in range(4):
                        nc.sync.dma_start(out=o_d[t, u], in_=osb[:, 128 * u : 128 * (u + 1)])
                rs_cur = rs_next
    nc.compile()
    return nc


_PROGRAMS = {}


def _get_programs():
    if "scan" not in _PROGRAMS:
        _PROGRAMS["scan"] = build_scan_program()
        _PROGRAMS["correct"] = build_correct_program()
    return _PROGRAMS["scan"], _PROGRAMS["correct"]


def _host_prep(recurrent_kernel):
    """R powers in fp64: R^1..R^16 stacked, and (R^16)^j for j=0..7, R^128."""
    R64 = recurrent_kernel.astype(np.float64)
    rp64 = []
    cur = np.eye(U)
    for _ in range(16):
        cur = cur @ R64
        rp64.append(cur)
    r16 = rp64[15]
    r16p64 = [np.eye(U)]
    for _ in range(7):
        r16p64.append(r16p64[-1] @ r16)
    r128_64 = r16p64[7] @ r16
    # rs layout: [4 kblk, 128 u', 16*512 (m-major cols)]
    rs = np.empty((4, 128, 16 * U), np.float32)
    for m in range(16):
        rp32 = rp64[m].astype(np.float32)
        for kb in range(4):
            rs[kb, :, m * U : (m + 1) * U] = rp32[128 * kb : 128 * (kb + 1), :]
    return rs, r16p64, r128_64


def kernel(x, h0, kernel, recurrent_kernel, _trace=False, _timings=None):
    x = np.ascontiguousarray(np.asarray(x), dtype=np.float32)
    h0 = np.asarray(h0).astype(np.float32)
    K = np.ascontiguousarray(np.asarray(kernel), dtype=np.float32)
    R = np.ascontiguousarray(np.asarray(recurrent_kernel), dtype=np.float32)

    scan_nc, corr_nc = _get_programs()
    rs, r16p64, r128_64 = _host_prep(R)

    trace_kw = dict(trace=True) if _trace else {}

    in1 = []
    for c in range(N_CORES):
        xt = np.ascontiguousarray(
            x[:, c * L : (c + 1) * L, :].transpose(2, 1, 0)
        ).reshape(4, 128, L, B)
        in1.append({"xt": xt, "k": K, "r": R})
    res1 = run_bass_kernel_spmd(scan_nc, in1, core_ids=list(range(N_CORES)), **trace_kw)
    if _timings is not None:
        _timings.append(res1.exec_time_ns)
    p_all = [res1.results[c]["p"] for c in range(N_CORES)]  # [L, 4, 128, B]

    # host boundary chain in fp64 (stays finite; fp32 casts reproduce inf/nan)
    g = h0.astype(np.float64)
    g_list = []
    for c in range(N_CORES):
        g_list.append(g)
        plast = p_all[c][L - 1]  # [4, 128u, 128b]
        hlast = np.concatenate([plast[u].T for u in range(4)], axis=1)  # [B, U]
        g = hlast.astype(np.float64) + g @ r128_64

    in2 = []
    for c in range(N_CORES):
        dT = np.empty((8, 4, 128, B), np.float32)
        for j in range(8):
            d32 = (g_list[c] @ r16p64[j]).astype(np.float32).T  # [U, B]
            for kb in range(4):
                dT[j, kb] = d32[128 * kb : 128 * (kb + 1), :]
        in2.append({"p": p_all[c], "dT": dT, "rs": rs})
    res2 = run_bass_kernel_spmd(corr_nc, in2, core_ids=list(range(N_CORES)), **trace_kw)
    if _timings is not None:
        _timings.append(res2.exec_time_ns)

    out = np.empty((B, T, U), np.float32)
    for c in range(N_CORES):
        o = res2.results[c]["o"]  # [L, 4, 128, B] u-major
        out[:, c * L : (c + 1) * L, :] = o.reshape(L, U, B).transpose(2, 0, 1)
    return out


# revision 8
# speedup vs baseline: 1.1974x; 1.1974x over previous
"""Trainium2 Bass kernel for MinimalRNNCell: h_t = x_t@K + h_{t-1}@R, outputs all h_t.

Shapes: x [128, 1024, 512], h0 [128, 512], K [512, 512], R [512, 512].
Sharding: TIME-sharded across 8 cores (each core owns a 128-step chunk with the
full batch of 128), exploiting linearity of the recurrence:
    h_{t0+i} = p_i + g @ R^{i+1}
where p is the zero-initialized local scan of the chunk and g = h_{t0-1} is the
chunk-entry state, obtained from a tiny 8-step boundary chain on the host.
This keeps every matmul full-tile (the batch-sharded alternative leaves only
16 batch rows per core, making every recurrence matmul issue-bound).

Launch 1 (per core): xk = x_chunk @ K fused into the 128-step local scan,
  all in u-major ("hT") orientation; writes p to HBM.
Host: boundary chain g_{c+1} = p_last_c + g_c @ R^128 (fp64), plus
  d_{c,j} = g_c @ (R^16)^j for j=0..7.
Launch 2 (per core): out_t = transpose(p_t) + d_j @ R^{m+1} for t=16j+m,
  with R^1..R^16 resident in SBUF; writes final [128, 128, 512] chunk.

All matmuls are true fp32 (4 cycles/row on the PE): the recurrence amplifies
matmul rounding error ~1000x over the sequence, so bf16/tf32 are not usable.
"""

import sys

if "/opt/trn_rl_repo" not in sys.path:
    sys.path.insert(0, "/opt/trn_rl_repo")

import numpy as np

import concourse.bacc as bacc
import concourse.bass as bass
import concourse.mybir as mybir
from concourse.bass_utils import run_bass_kernel_spmd
from concourse.tile import TileContext

N_CORES = 8
B, T, D, U = 128, 1024, 512, 512
L = T // N_CORES  # 128 timesteps per core
NG = L // 4  # 4-step groups in the local scan
FP32 = mybir.dt.float32


def _patch_tile_drain(max_waits=1):
    """Walrus in this image rejects >1 sem wait on a CTRL (Drain) inst; split
    the TileContext tail drain into one drain per pending proc."""
    from concourse import tile
    from concourse.vector_clock import ScopedClock, VectorClock

    if getattr(tile.TileContext, "_drain_patched", False):
        return

    def _drain_and_barrier(self, tick_clock, wait_clock):
        nc = self.nc
        gc = tick_clock.global_clock
        nonzero = [(i, gc[i]) for i in range(len(gc)) if gc[i] > 0]
        for g in range(0, len(nonzero), max_waits):
            sub = VectorClock()
            for i, t in nonzero[g : g + max_waits]:
                sub.require_at_least(i, t)
            d = nc.sync.drain()
            wait_clock.add_sem_waits(d.ins, ScopedClock({None: sub}))
        nc.all_engine_barrier()
        assert self.sems is not None
        popped = nc._tile_sem_poison_stack.pop()
        assert popped is self._sem_poison
        nc.clear_and_free_semaphores(list(self.sems.allocated().values()))
        nc.all_engine_barrier()

    tile.TileContext._drain_and_barrier = _drain_and_barrier
    tile.TileContext._drain_patched = True


def build_scan_program():
    """Launch 1: fused xk GEMM + local scan, u-major. Per core:
    in:  xt [4dblk, 128d, 128t, 128b] (host-pretransposed x chunk),
         K [512, 512], R [512, 512]
    out: p [128t, 4ublk, 128u, 128b]  (hT tiles of the zero-init local scan)
    """
    nc = bacc.Bacc("TRN2", target_bir_lowering=False, debug=False, num_devices=N_CORES)
    xt_d = nc.dram_tensor("xt", [4, 128, L, B], FP32, kind="ExternalInput")
    k_d = nc.dram_tensor("k", [D, U], FP32, kind="ExternalInput")
    r_d = nc.dram_tensor("r", [U, U], FP32, kind="ExternalInput")
    p_d = nc.dram_tensor("p", [L, 4, 128, B], FP32, kind="ExternalOutput")

    with TileContext(nc) as tc:
        with (
            tc.tile_pool(name="const", bufs=1) as cpool,
            tc.tile_pool(name="xt", bufs=3) as xtpool,
            tc.tile_pool(name="ht", bufs=4) as htpool,
            tc.tile_pool(name="psum", bufs=2, space="PSUM") as ppool,
        ):
            k_sb = []
            r_sb = []
            for i in range(4):
                kt = cpool.tile([128, U], FP32, tag=f"k{i}")
                nc.sync.dma_start(out=kt[:], in_=k_d[128 * i : 128 * (i + 1), :])
                k_sb.append(kt)
                rt = cpool.tile([128, U], FP32, tag=f"r{i}")
                nc.sync.dma_start(out=rt[:], in_=r_d[128 * i : 128 * (i + 1), :])
                r_sb.append(rt)
            h_zero = []
            for i in range(4):
                z = cpool.tile([128, B], FP32, tag=f"z{i}")
                nc.gpsimd.memset(z[:], 0.0)
                h_zero.append(z)

            def load_group(g):
                xt = [xtpool.tile([128, 4, 128], FP32, tag=f"xt{d}", name=f"xt{d}") for d in range(4)]
                for d in range(4):
                    nc.sync.dma_start(out=xt[d][:], in_=xt_d[d, :, 4 * g : 4 * g + 4, :])
                return xt

            h_prev = h_zero
            xt_cur = load_group(0)
            for g in range(NG):
                xt_next = load_group(g + 1) if g + 1 < NG else None
                ph = [ppool.tile([128, 512], FP32, tag=f"ph{u}", name=f"ph{u}") for u in range(4)]
                for u in range(4):
                    for d in range(4):
                        nc.tensor.matmul(
                            ph[u][:],
                            k_sb[d][:, 128 * u : 128 * (u + 1)],
                            xt_cur[d][:],
                            start=(d == 0),
                            stop=False,
                        )
                for s in range(4):
                    i = 4 * g + s
                    sl = slice(128 * s, 128 * (s + 1))
                    h_new = []
                    for u in range(4):
                        for kb in range(4):
                            nc.tensor.matmul(
                                ph[u][:, sl],
                                r_sb[kb][:, 128 * u : 128 * (u + 1)],
                                h_prev[kb][:],
                                start=False,
                                stop=(kb == 3),
                            )
                        ht = htpool.tile([128, B], FP32, tag=f"ht{u}", name=f"ht{u}")
                        nc.vector.tensor_copy(ht[:], ph[u][:, sl])
                        nc.sync.dma_start(out=p_d[i, u], in_=ht[:])
                        h_new.append(ht)
                    h_prev = h_new
                xt_cur = xt_next
    nc.compile()
    return nc


def build_correct_program():
    """Launch 2: out_t = transpose(p_t) + d_j @ R^{m+1}, t = 16j+m (B-major out).
    Per core:
    in:  p [128t, 4ublk, 128u, 128b], dT [8j, 4kblk, 128u', 128b],
         rs [4kblk, 128u', 16*512 (m-major)], eye [128, 128]
    out: o [128b, 128t, 512u]
    """
    nc = bacc.Bacc("TRN2", target_bir_lowering=False, debug=False, num_devices=N_CORES)
    p_d = nc.dram_tensor("p", [L, 4, 128, B], FP32, kind="ExternalInput")
    dt_d = nc.dram_tensor("dT", [8, 4, 128, B], FP32, kind="ExternalInput")
    rs_d = nc.dram_tensor("rs", [4, 128, 16 * U], FP32, kind="ExternalInput")
    eye_d = nc.dram_tensor("eye", [128, 128], FP32, kind="ExternalInput")
    o_d = nc.dram_tensor("o", [B, L, U], FP32, kind="ExternalOutput")

    with TileContext(nc) as tc:
        with (
            tc.tile_pool(name="const", bufs=1) as cpool,
            tc.tile_pool(name="rs", bufs=2) as rspool,
            tc.tile_pool(name="pin", bufs=6) as pinpool,
            tc.tile_pool(name="os", bufs=6) as ospool,
            tc.tile_pool(name="psum", bufs=2, space="PSUM") as ppool,
        ):
            dt_sb = {}
            for j in range(8):
                for kb in range(4):
                    dtt = cpool.tile([128, B], FP32, tag=f"dt{j}_{kb}", name=f"dt{j}_{kb}")
                    nc.sync.dma_start(out=dtt[:], in_=dt_d[j, kb])
                    dt_sb[(j, kb)] = dtt
            eye = cpool.tile([128, 128], FP32, tag="eye")
            nc.sync.dma_start(out=eye[:], in_=eye_d[:])

            def load_rs(m):
                rs = [rspool.tile([128, U], FP32, tag=f"rs{kb}", name=f"rs{kb}") for kb in range(4)]
                for kb in range(4):
                    nc.sync.dma_start(out=rs[kb][:], in_=rs_d[kb, :, m * U : (m + 1) * U])
                return rs

            rs_cur = load_rs(0)
            for m in range(16):
                rs_next = load_rs(m + 1) if m + 1 < 16 else None
                for j in range(8):
                    t = 16 * j + m
                    pin = [pinpool.tile([128, B], FP32, tag=f"pin{u}", name=f"pin{u}") for u in range(4)]
                    for u in range(4):
                        nc.sync.dma_start(out=pin[u][:], in_=p_d[t, u])
                    po = ppool.tile([128, U], FP32, tag=f"po{j % 2}", name=f"po{j % 2}")
                    for kb in range(4):
                        nc.tensor.matmul(
                            po[:],
                            dt_sb[(j, kb)][:],
                            rs_cur[kb][:],
                            start=(kb == 0),
                            stop=False,
                        )
                    for u in range(4):
                        nc.tensor.matmul(
                            po[:, 128 * u : 128 * (u + 1)],
                            pin[u][:],
                            eye[:],
                            is_transpose=True,
                            start=False,
                            stop=True,
                        )
                    osb = ospool.tile([128, U], FP32, tag="os", name="os")
                    nc.vector.tensor_copy(osb[:], po[:])
                    nc.sync.dma_start(out=o_d[:, t, :], in_=osb[:])
                rs_cur = rs_next
    nc.compile()
    return nc


_PROGRAMS = {}


def _get_programs():
    if "scan" not in _PROGRAMS:
        _PROGRAMS["scan"] = build_scan_program()
        _PROGRAMS["correct"] = build_correct_program()
    return _PROGRAMS["scan"], _PROGRAMS["correct"]


def _host_prep(recurrent_kernel):
    """R powers in fp64: R^1..R^16 stacked, and (R^16)^j for j=0..7, R^128."""
    R64 = recurrent_kernel.astype(np.float64)
    rp64 = []
    cur = np.eye(U)
    for _ in range(16):
        cur = cur @ R64
        rp64.append(cur)
    r16 = rp64[15]
    r16p64 = [np.eye(U)]
    for _ in range(7):
        r16p64.append(r16p64[-1] @ r16)
    r128_64 = r16p64[7] @ r16
    # rs layout: [4 kblk, 128 u', 16*512 (m-major cols)]
    rs = np.empty((4, 128, 16 * U), np.float32)
    for m in range(16):
        rp32 = rp64[m].astype(np.float32)
        for kb in range(4):
            rs[kb, :, m * U : (m + 1) * U] = rp32[128 * kb : 128 * (kb + 1), :]
    return rs, r16p64, r128_64


def kernel(x, h0, kernel, recurrent_kernel, _trace=False, _timings=None):
    x = np.ascontiguousarray(np.asarray(x), dtype=np.float32)
    h0 = np.asarray(h0).astype(np.float32)
    K = np.ascontiguousarray(np.asarray(kernel), dtype=np.float32)
    R = np.ascontiguousarray(np.asarray(recurrent_kernel), dtype=np.float32)
    eye = np.eye(128, dtype=np.float32)

    scan_nc, corr_nc = _get_programs()
    rs, r16p64, r128_64 = _host_prep(R)

    trace_kw = dict(trace=True) if _trace else {}

    in1 = []
    for c in range(N_CORES):
        xt = np.ascontiguousarray(
            x[:, c * L : (c + 1) * L, :].transpose(2, 1, 0)
        ).reshape(4, 128, L, B)
        in1.append({"xt": xt, "k": K, "r": R})
    res1 = run_bass_kernel_spmd(scan_nc, in1, core_ids=list(range(N_CORES)), **trace_kw)
    if _timings is not None:
        _timings.append(res1.exec_time_ns)
    p_all = [res1.results[c]["p"] for c in range(N_CORES)]  # [L, 4, 128, B]

    # host boundary chain in fp64 (stays finite; fp32 casts reproduce inf/nan)
    g = h0.astype(np.float64)
    g_list = []
    for c in range(N_CORES):
        g_list.append(g)
        plast = p_all[c][L - 1]  # [4, 128u, 128b]
        hlast = np.concatenate([plast[u].T for u in range(4)], axis=1)  # [B, U]
        g = hlast.astype(np.float64) + g @ r128_64

    in2 = []
    for c in range(N_CORES):
        dT = np.empty((8, 4, 128, B), np.float32)
        for j in range(8):
            d32 = (g_list[c] @ r16p64[j]).astype(np.float32).T  # [U, B]
            for kb in range(4):
                dT[j, kb] = d32[128 * kb : 128 * (kb + 1), :]
        in2.append({"p": p_all[c], "dT": dT, "rs": rs, "eye": eye})
    res2 = run_bass_kernel_spmd(corr_nc, in2, core_ids=list(range(N_CORES)), **trace_kw)
    if _timings is not None:
        _timings.append(res2.exec_time_ns)

    out = np.empty((B, T, U), np.float32)
    for c in range(N_CORES):
        out[:, c * L : (c + 1) * L, :] = res2.results[c]["o"]
    return out
